# revision 1
# baseline (speedup 1.0000x reference)
"""Causal self-attention (B=4, T=2048, C=1024, H=16) on 8 trn2 NeuronCores.

Sharding: tensor-parallel over heads — each core owns 2 heads (128 of the
1024 channel dims). Each core computes its Q/K/V slices from the full x,
runs causal attention for its heads over all batches, and produces a
partial output projection; the host sums the 8 partials (the all-reduce).

Layout trick: attention scores are computed transposed (S^T[tk, tq]) so
softmax needs no on-chip transposes anywhere in the inner loop:
  - S^T = kT.T @ qT              (kT/qT are [head_dim, tokens] in SBUF)
  - P = exp(S^T)                 (no row-max: scores ~ N(0,1), exp is safe)
  - out[tq, d] = P.T @ v_aug     (v_aug has a ones column -> denominator)
  - normalize with a per-partition scalar multiply (tq is the partition dim)
Causality = skip fully-invalid blocks + one 128x128 triangle mask multiply
on the diagonal block.

Schedule: the QKV projection work for token-chunk i+1 is spliced into the
attention j-loop of chunk i so the TensorE keeps busy while ScalarE works
through the exp() backlog (exp is the second-largest engine load).
"""

import sys

if "/opt/trn_rl_repo" not in sys.path:
    sys.path.insert(0, "/opt/trn_rl_repo")

import ml_dtypes
import numpy as np

B, T, C, H = 4, 2048, 1024, 16
HD = C // H          # 64
NCORES = 8
HPC = H // NCORES    # heads per core = 2
DPC = HPC * HD       # channel dims per core = 128
N = B * T            # 8192 tokens
P = 128              # partitions
TCH = 512            # token chunk (psum bank width in fp32)
KB = C // P          # contraction blocks in stage 1 = 8
NTCH = N // TCH      # 16 token chunks overall
NQC = T // TCH       # tq chunks per batch = 4
NTB = T // P         # 128-token blocks per batch = 16

BF16 = ml_dtypes.bfloat16
REPEAT = 1           # >1 wraps the body in a hardware loop (for benchmarking)
ABLATE = set()       # dev-only: {"exp","av","st","proj","dmaout","s1"}

_CACHE = {}


def _build_nc(repeat=None):
    import concourse.tile as tile
    from concourse import bacc, mybir

    repeat = REPEAT if repeat is None else repeat
    nc = bacc.Bacc(None, target_bir_lowering=False)
    f32 = mybir.dt.float32
    bf16 = mybir.dt.bfloat16
    AF = mybir.ActivationFunctionType

    # ---- DRAM I/O (per-core tensors; same program on all 8 cores) ----
    xt_d = nc.dram_tensor("xt", [C, N], bf16, kind="ExternalInput")
    wq_d = nc.dram_tensor("wq", [C, DPC], bf16, kind="ExternalInput")
    wk_d = nc.dram_tensor("wk", [C, DPC], bf16, kind="ExternalInput")
    wv_d = nc.dram_tensor("wv", [C, DPC], bf16, kind="ExternalInput")
    wp_d = nc.dram_tensor("wp", [DPC, C], bf16, kind="ExternalInput")
    bq_d = nc.dram_tensor("bq", [DPC, 1], f32, kind="ExternalInput")
    bk_d = nc.dram_tensor("bk", [DPC, 1], f32, kind="ExternalInput")
    bv_d = nc.dram_tensor("bv", [P, DPC], f32, kind="ExternalInput")
    tri_d = nc.dram_tensor("tri", [P, P], bf16, kind="ExternalInput")
    id_d = nc.dram_tensor("idn", [P, P], bf16, kind="ExternalInput")
    out_d = nc.dram_tensor("out", [N, C], bf16, kind="ExternalOutput")

    with tile.TileContext(nc) as tc:
        with (
            tc.tile_pool(name="persist", bufs=1) as persist,
            tc.tile_pool(name="xp", bufs=16) as xp,
            tc.tile_pool(name="ptp", bufs=12) as ptp,
            tc.tile_pool(name="ysp", bufs=10) as ysp,
            tc.tile_pool(name="ytp", bufs=4) as ytp,
            tc.tile_pool(name="osp", bufs=3) as osp,
            tc.tile_pool(name="rcp", bufs=8) as rcp,
            tc.tile_pool(name="s1p", bufs=2, space="PSUM") as s1p,
            tc.tile_pool(name="big", bufs=4, space="PSUM") as big,
            tc.tile_pool(name="avp", bufs=2, space="PSUM") as avp,
        ):
            # ---- persistent SBUF ----
            qTs = persist.tile([P, N], bf16, tag="qTs")   # [dims, tokens]
            kTs = persist.tile([P, N], bf16, tag="kTs")
            # v blocks: per 128-token block: [v_h0 | 1 | v_h1 | 1] = 130 cols
            vs = persist.tile([P, (N // P) * 130], bf16, tag="vs")
            wqs = persist.tile([P, C], bf16, tag="wqs")   # 8 blocks of [128,128]
            wks = persist.tile([P, C], bf16, tag="wks")
            wvs = persist.tile([P, C], bf16, tag="wvs")
            wps = persist.tile([P, C], bf16, tag="wps")
            bqs = persist.tile([P, 1], f32, tag="bqs")
            bks = persist.tile([P, 1], f32, tag="bks")
            bvs = persist.tile([P, DPC], f32, tag="bvs")
            tri = persist.tile([P, P], bf16, tag="tri")
            idn = persist.tile([P, P], bf16, tag="idn")

            for k in range(KB):
                nc.sync.dma_start(out=wqs[:, k * P:(k + 1) * P],
                                  in_=wq_d[k * P:(k + 1) * P, :])
                nc.sync.dma_start(out=wks[:, k * P:(k + 1) * P],
                                  in_=wk_d[k * P:(k + 1) * P, :])
                nc.sync.dma_start(out=wvs[:, k * P:(k + 1) * P],
                                  in_=wv_d[k * P:(k + 1) * P, :])
            nc.sync.dma_start(out=wps[:, :], in_=wp_d[:, :])
            nc.sync.dma_start(out=bqs[:, :], in_=bq_d[:, :])
            nc.sync.dma_start(out=bks[:, :], in_=bk_d[:, :])
            nc.sync.dma_start(out=bvs[:, :], in_=bv_d[:, :])
            nc.sync.dma_start(out=tri[:, :], in_=tri_d[:, :])
            nc.sync.dma_start(out=idn[:, :], in_=id_d[:, :])

            vs_r = vs.rearrange("p (t c) -> p t c", c=130)
            nc.vector.memset(vs_r[:, :, HD:HD + 1], 1.0)
            nc.vector.memset(vs_r[:, :, 2 * HD + 1:2 * HD + 2], 1.0)

            # ---------------- body ----------------
            def s1_units(tch):
                """QKV projection for token chunk `tch`, as a list of work
                units (callables) to be spliced between attention steps."""
                if tch >= NTCH:
                    return []
                t0 = tch * TCH
                xts = []

                def load():
                    for k in range(KB):
                        xtile = xp.tile([P, TCH], bf16, tag="xt",
                                        name=f"xt{tch}_{k}")
                        nc.sync.dma_start(
                            out=xtile, in_=xt_d[k * P:(k + 1) * P, t0:t0 + TCH])
                        xts.append(xtile)
                state = {}

                def qk_mms(k):
                    def f():
                        if k == 0:
                            state["psq"] = s1p.tile([P, TCH], f32, tag="s1",
                                                    name=f"psq{tch}")
                            state["psk"] = s1p.tile([P, TCH], f32, tag="s1",
                                                    name=f"psk{tch}")
                        nc.tensor.matmul(state["psq"][:],
                                         wqs[:, k * P:(k + 1) * P], xts[k][:],
                                         start=(k == 0), stop=(k == KB - 1))
                        nc.tensor.matmul(state["psk"][:],
                                         wks[:, k * P:(k + 1) * P], xts[k][:],
                                         start=(k == 0), stop=(k == KB - 1))
                        if k == KB - 1:
                            nc.vector.tensor_scalar_add(qTs[:, t0:t0 + TCH],
                                                        state["psq"][:],
                                                        bqs[:, :])
                            nc.vector.tensor_scalar_add(kTs[:, t0:t0 + TCH],
                                                        state["psk"][:],
                                                        bks[:, :])
                    return f

                def v_mms(m):
                    def f():
                        tb = (t0 + m * P) // P
                        psv = big.tile([P, P], f32, tag="big", name=f"psv{tch}_{m}")
                        for k in range(KB):
                            nc.tensor.matmul(psv[:],
                                             xts[k][:, m * P:(m + 1) * P],
                                             wvs[:, k * P:(k + 1) * P],
                                             start=(k == 0), stop=(k == KB - 1))
                        for h in range(HPC):
                            nc.vector.tensor_add(
                                vs_r[:, tb, h * (HD + 1):h * (HD + 1) + HD],
                                psv[:, h * HD:(h + 1) * HD],
                                bvs[:, h * HD:(h + 1) * HD])
                    return f

                units = [load]
                for k in range(KB):
                    units.append(qk_mms(k))
                for m in range(TCH // P):
                    units.append(v_mms(m))
                return units

            def att_chunk(b, c, splice):
                """Attention for tq chunk c of batch b, with `splice` work
                units interleaved into the j loop."""
                base = b * T
                q0 = base + c * TCH
                J = 4 * c + 4
                ys_tiles = []
                for m in range(NQC):
                    yt_ = ysp.tile([P, P], bf16, tag="ys", name=f"ys{b}_{c}_{m}")
                    ys_tiles.append(yt_)
                avpair = [avp.tile([P, 4 * (HD + 1)], f32, tag="av",
                                   name=f"avpair_{b}_{c}_{p}")
                          for p in range(NQC // 2)]
                av2 = [avpair[m // 2][:, (m % 2) * 2 * (HD + 1):
                                      (m % 2 + 1) * 2 * (HD + 1)]
                       for m in range(NQC)]
                pts = {}
                si = 0
                nsplice = len(splice)

                def do_splice(upto):
                    nonlocal si
                    while si < min(upto, nsplice):
                        splice[si]()
                        si += 1

                LAG = 4
                for j in range(J + LAG):
                    if j < J:
                        r = j - 4 * c
                        u0 = max(0, r) * P
                        for h in range(HPC):
                            hq = h * HD
                            st = big.tile([P, TCH], f32, tag="big",
                                          name=f"st{b}_{c}_{j}_{h}")
                            if "st" in ABLATE:
                                nc.tensor.matmul(
                                    st[:, u0:u0 + 1],
                                    kTs[hq:hq + HD, base + j * P:base + (j + 1) * P],
                                    qTs[hq:hq + HD, q0 + u0:q0 + u0 + 1],
                                    start=True, stop=True)
                            else:
                                nc.tensor.matmul(
                                    st[:, u0:TCH],
                                    kTs[hq:hq + HD, base + j * P:base + (j + 1) * P],
                                    qTs[hq:hq + HD, q0 + u0:q0 + TCH],
                                    start=True, stop=True)
                            pt = ptp.tile([P, TCH], bf16, tag="pt",
                                          name=f"pt{b}_{c}_{j}_{h}")
                            if "exp" in ABLATE:
                                nc.scalar.activation(pt[:, u0:u0 + 1],
                                                     st[:, u0:u0 + 1], AF.Exp)
                            else:
                                nc.scalar.activation(pt[:, u0:TCH], st[:, u0:TCH],
                                                     AF.Exp)
                            if r >= 0:
                                nc.gpsimd.tensor_mul(pt[:, u0:u0 + P],
                                                     pt[:, u0:u0 + P], tri[:])
                            pts[(j, h)] = pt
                    # splice stage-1 work for the next chunk between steps
                    do_splice((j + 1) * nsplice // (J + LAG))
                    if j >= LAG:
                        jj = j - LAG
                        for h in range(HPC):
                            pt = pts.pop((jj, h))
                            for m in range(NQC):
                                if jj > 4 * c + m:
                                    continue
                                vtb = b * NTB + jj
                                nn = 1 if "av" in ABLATE else HD + 1
                                nc.tensor.matmul(
                                    av2[m][:, h * (HD + 1):h * (HD + 1) + nn],
                                    pt[:, m * P:(m + 1) * P],
                                    vs_r[:, vtb, h * (HD + 1):h * (HD + 1) + nn],
                                    start=(jj == 0 and h == 0 and m % 2 == 0),
                                    stop=(h == 1 and m % 2 == 1
                                          and jj == 4 * c + m))
                        # normalize a pair as soon as its accumulation group
                        # closed (group spans both m of the pair), so the psum
                        # bank frees before the chunk tail
                        for m in range(NQC):
                            if jj != 4 * c + (m | 1):
                                continue
                            for h in range(HPC):
                                o = h * (HD + 1)
                                rec = rcp.tile([P, 1], f32, tag="rec",
                                               name=f"rec{b}_{c}_{m}_{h}")
                                nc.vector.reciprocal(
                                    rec[:], av2[m][:, o + HD:o + HD + 1])
                                nc.vector.tensor_scalar_mul(
                                    ys_tiles[m][:, h * HD:(h + 1) * HD],
                                    av2[m][:, o:o + HD], rec[:])
                do_splice(nsplice)
                return ys_tiles

            def proj_units(b, c, ys_tiles):
                if "proj" in ABLATE:
                    return []
                base = b * T
                units = []
                for m in range(NQC):
                    state = {}

                    def tr_unit(m=m, state=state):
                        tr_ps = big.tile([P, P], bf16, tag="big",
                                         name=f"tr{b}_{c}_{m}")
                        nc.tensor.transpose(tr_ps[:], ys_tiles[m][:], idn[:])
                        yt = ytp.tile([P, P], bf16, tag="yt",
                                      name=f"yt{b}_{c}_{m}")
                        nc.vector.tensor_copy(yt[:], tr_ps[:])
                        state["yt"] = yt
                        state["osb"] = osp.tile([P, C], bf16, tag="os",
                                                name=f"os{b}_{c}_{m}")

                    def mm_unit(oc, m=m, state=state):
                        def f():
                            pp = big.tile([P, TCH], f32, tag="big",
                                          name=f"pp{b}_{c}_{m}_{oc}")
                            nc.tensor.matmul(pp[:], state["yt"][:],
                                             wps[:, oc * TCH:(oc + 1) * TCH],
                                             start=True, stop=True)
                            if oc == 0:
                                nc.scalar.copy(
                                    state["osb"][:, 0:TCH], pp[:])
                            else:
                                nc.vector.tensor_copy(
                                    state["osb"][:, TCH:C], pp[:])
                            if oc == 1:
                                row0 = base + (c * 4 + m) * P
                                if "dmaout" not in ABLATE:
                                    nc.sync.dma_start(
                                        out=out_d[row0:row0 + P, :],
                                        in_=state["osb"][:])
                        return f

                    units += [tr_unit, mm_unit(0), mm_unit(1)]
                return units

            def body():
                for u in s1_units(0):
                    u()
                pending_proj = []
                for i in range(NTCH):
                    b, c = divmod(i, NQC)
                    splice = pending_proj + s1_units(i + 1)
                    ys_tiles = att_chunk(b, c, splice)
                    pending_proj = proj_units(b, c, ys_tiles)
                for u in pending_proj:
                    u()

            if repeat > 1:
                from concourse import mybir as _mb
                with tc.For_i(0, repeat, 1, hint_engines=(
                        _mb.EngineType.PE, _mb.EngineType.Activation,
                        _mb.EngineType.DVE, _mb.EngineType.SP,
                        _mb.EngineType.Pool)):
                    body()
            else:
                body()
    nc.compile()
    return nc


def _get_nc():
    if "nc" not in _CACHE:
        _CACHE["nc"] = _build_nc()
    return _CACHE["nc"]


def _make_in_maps(x, Wk, bk, Wq, bq, Wv, bv, Wp, bp):
    x2 = np.ascontiguousarray(np.asarray(x, np.float32).reshape(N, C).T)
    xt = x2.astype(BF16)
    scale = 1.0 / np.sqrt(HD)
    wqt = (np.asarray(Wq, np.float32).T * scale).astype(BF16)
    wkt = np.asarray(Wk, np.float32).T.astype(BF16)
    wvt = np.asarray(Wv, np.float32).T.astype(BF16)
    wpt = np.asarray(Wp, np.float32).T.astype(BF16)
    tri = np.triu(np.ones((P, P), np.float32)).astype(BF16)
    idn = np.eye(P, dtype=np.float32).astype(BF16)
    in_maps = []
    for cidx in range(NCORES):
        s = slice(cidx * DPC, (cidx + 1) * DPC)
        in_maps.append({
            "xt": xt,
            "wq": np.ascontiguousarray(wqt[:, s]),
            "wk": np.ascontiguousarray(wkt[:, s]),
            "wv": np.ascontiguousarray(wvt[:, s]),
            "wp": np.ascontiguousarray(wpt[s, :]),
            "bq": (np.asarray(bq, np.float32)[s] * scale).reshape(DPC, 1),
            "bk": np.asarray(bk, np.float32)[s].reshape(DPC, 1),
            "bv": np.ascontiguousarray(np.broadcast_to(
                np.asarray(bv, np.float32)[s], (P, DPC))),
            "tri": tri,
            "idn": idn,
        })
    return in_maps


def kernel(x, Wk, bk, Wq, bq, Wv, bv, Wp, bp):
    from concourse.bass_utils import run_bass_kernel_spmd

    nc = _get_nc()
    in_maps = _make_in_maps(x, Wk, bk, Wq, bq, Wv, bv, Wp, bp)
    res = run_bass_kernel_spmd(nc, in_maps, core_ids=list(range(NCORES)))
    acc = np.zeros((N, C), np.float64)
    for r in res.results:
        acc += r["out"].astype(np.float64)
    out = (acc + np.asarray(bp, np.float64)).astype(np.float32)
    return out.reshape(B, T, C)

